# revision 26
# baseline (speedup 1.0000x reference)
"""Trainium2 Bass kernel for AdaptConv-style GNN message passing.

Reference computation (per batch element b):
    h   = x @ W.T + b                       # [N, OUT]
    hn  = h / max(||h||_row, 1e-12)         # row-wise L2 normalize
    cos = hn @ hn.T                         # [N, N]
    out = relu((edge_weight * cos) @ h)     # [N, OUT]

Sharding: pure data-parallel over batch B=8 across the 8 NeuronCores
(no collectives).  Host-side layout preprocessing (part of the sharding
strategy): each core receives
    et   = edge_weight[b].T (bf16)  [N, N]  (so the gated matrix is produced
                                             directly in the [q, p] layout the
                                             aggregation matmul contracts over;
                                             bf16 halves the HBM stream and is
                                             well inside the accuracy budget)
    xt   = x[b].T               [IN, N]
    wt   = W.T                  [IN, OUT]
    bias = b.reshape(OUT, 1)
and returns outT = relu(out).T as [OUT, N]; the host transposes back.

On-chip dataflow per core (matmuls bf16, fp32 PSUM accumulation):
    hT[o, n]   = wt.T @ xt + bias          (TensorE + ScalarE bias)
    h_rm tiles = PE-transpose(hT)          (row-major h, agg stationary)
    norms      = Square+accum (ScalarE/VectorE) -> Sqrt/max/recip [128,16]
    hnT        = PE-transpose(h_rm * r)
    4 column passes x 16 q-bands (et bf16 fully SBUF-resident):
        cosT[q', p'] = hnT[:, q]^T @ hnT[:, sl]      (PE -> PSUM)
        gT           = et[q, sl] * cosT              (DVE / ACT+GpSimd / ACT+DVE)
        outT[:, sl] += h_rm[q]^T @ gT                (PE, PSUM accum, LAG behind)
    relu epilogue per pass (ScalarE) + DMA out.

PSUM: pass-local outT = 1 bank (2 alternating) + 6-deep cos pipeline = 8.
"""

import ml_dtypes
import numpy as np

import concourse.bass as bass
import concourse.mybir as mybir
import concourse.tile as tile
from concourse import bacc
from concourse.bass_utils import run_bass_kernel_spmd
from concourse.masks import make_identity

B, N, IN, OUT = 8, 2048, 128, 128
NQ = N // 128
NPC = N // 512
FP32 = mybir.dt.float32
BF16 = mybir.dt.bfloat16
AF = mybir.ActivationFunctionType
EPS = 1e-12

CORE_IDS = list(range(8))


def build_nc():
    """Build + compile the single-core Bass graph (same graph runs SPMD on 8 cores)."""
    from contextlib import ExitStack

    nc = bacc.Bacc("TRN2", target_bir_lowering=False, debug=False, num_devices=8)

    et = nc.dram_tensor("et", [N, N], BF16, kind="ExternalInput").ap()
    xt = nc.dram_tensor("xt", [IN, N], FP32, kind="ExternalInput").ap()
    wt = nc.dram_tensor("wt", [IN, OUT], FP32, kind="ExternalInput").ap()
    bias = nc.dram_tensor("bias", [OUT, 1], FP32, kind="ExternalInput").ap()
    out = nc.dram_tensor("out", [OUT, N], FP32, kind="ExternalOutput").ap()

    with tile.TileContext(nc) as tc, ExitStack() as ctx:
        singles = ctx.enter_context(tc.tile_pool(name="singles", bufs=1))
        # et stream pool FIRST so its SBUF range never aliases prologue
        # scratch (WAR deps would stall the stream behind the prologue).
        etp = ctx.enter_context(tc.tile_pool(name="etp", bufs=16))
        gtp = ctx.enter_context(tc.tile_pool(name="gtp", bufs=5))
        csp = ctx.enter_context(tc.tile_pool(name="csp", bufs=3))

        ident = singles.tile([128, 128], BF16, tag="ident")
        make_identity(nc, ident[:])

        hnT = singles.tile([128, N], BF16, tag="hnT")
        hrm = [
            singles.tile([128, OUT], BF16, tag=f"hrm{i}", name=f"hrm{i}")
            for i in range(NQ)
        ]
        out_sb = singles.tile([OUT, N], FP32, tag="out_sb")
        bias_sb = singles.tile([OUT, 1], FP32, tag="bias")
        s_acc = singles.tile([128, NQ], FP32, tag="s_acc")
        s_nrm = singles.tile([128, NQ], FP32, tag="s_nrm")
        s_max = singles.tile([128, NQ], FP32, tag="s_max")
        r_inv = singles.tile([128, NQ], FP32, tag="r_inv")

        # params first on the sync ring (xt heads the prologue critical
        # path), then the et stream queues right behind
        xt_f = singles.tile([IN, N], FP32, tag="xt_f")
        nc.sync.dma_start(xt_f[:], xt)
        wt_f = singles.tile([IN, OUT], FP32, tag="wt_f")
        nc.sync.dma_start(wt_f[:], wt)
        nc.sync.dma_start(bias_sb[:], bias)

        # et stream: all 16 bands prefetched into SBUF (bf16, 64KB/partition)
        etbs = []
        for q in range(NQ):
            etb = etp.tile([128, N], BF16, tag="etb", name=f"etb{q}")
            nc.sync.dma_start(etb[:], et[q * 128 : (q + 1) * 128, :])
            etbs.append(etb)

        # ---------------- prologue: h, norms, hn (scoped pools) ----------------
        with ExitStack() as pctx:
            pro = pctx.enter_context(tc.tile_pool(name="pro", bufs=2))
            ppsum = pctx.enter_context(tc.tile_pool(name="ppsum", bufs=2, space="PSUM"))

            # warm the ScalarE Sqrt activation table off the critical path
            dummy = pro.tile([1, 2], FP32, tag="dummy")
            nc.gpsimd.memset(dummy[:], 1.0)
            dummy2 = pro.tile([1, 2], FP32, tag="dummy2")
            nc.scalar.activation(dummy2[:], dummy[:], AF.Sqrt)

            wt_b = pro.tile([IN, OUT], BF16, tag="wt_b")
            nc.vector.tensor_copy(wt_b[:], wt_f[:])
            # cast + linear per 512-chunk so compute starts before the full
            # xt transfer lands
            xt_b = pro.tile([IN, N], BF16, tag="xt_b")
            hT = pro.tile([128, N], BF16, tag="hT")
            for c in range(N // 512):
                sl = slice(c * 512, (c + 1) * 512)
                nc.vector.tensor_copy(xt_b[:, sl], xt_f[:, sl])
                ps = ppsum.tile([OUT, 512], FP32, tag="hT_ps")
                nc.tensor.matmul(ps[:], wt_b[:], xt_b[:, sl], start=True, stop=True)
                # hT = psum + bias (per-partition bias along OUT)
                nc.scalar.activation(
                    hT[:, sl], ps[:], AF.Identity, bias=bias_sb[:], scale=1.0
                )

            # stage 1: transposes hT -> row-major h tiles; square+rowsum
            # (norm reductions split across ScalarE / VectorE)
            for i in range(NQ):
                tp = ppsum.tile([128, 128], BF16, tag="tp", bufs=4)
                nc.tensor.transpose(tp[:], hT[:, i * 128 : (i + 1) * 128], ident[:])
                nc.any.tensor_copy(hrm[i][:], tp[:])
                sq = pro.tile([128, OUT], BF16, tag="sq", bufs=4)
                if i % 2 == 0:
                    nc.scalar.activation(
                        sq[:], hrm[i][:], AF.Square, accum_out=s_acc[:, i : i + 1]
                    )
                else:
                    nc.vector.tensor_mul(sq[:], hrm[i][:], hrm[i][:])
                    nc.vector.tensor_reduce(
                        s_acc[:, i : i + 1], sq[:],
                        mybir.AxisListType.X, mybir.AluOpType.add,
                    )
            # stage 2: batched sqrt -> clamp -> reciprocal
            nc.scalar.activation(s_nrm[:], s_acc[:], AF.Sqrt)
            nc.vector.tensor_scalar_max(s_max[:], s_nrm[:], EPS)
            nc.vector.reciprocal(r_inv[:], s_max[:])

            # stage 3: hn tiles + transpose back into hnT
            for i in range(NQ):
                hn_i = pro.tile([128, OUT], BF16, tag="hn_i", bufs=4)
                nc.vector.tensor_scalar_mul(hn_i[:], hrm[i][:], r_inv[:, i : i + 1])
                tp2 = ppsum.tile([128, 128], BF16, tag="tp", bufs=4)
                nc.tensor.transpose(tp2[:], hn_i[:], ident[:])
                nc.any.tensor_copy(hnT[:, i * 128 : (i + 1) * 128], tp2[:])

        # ---------------- main loop: 2 column passes x 16 bands ----------------
        # 1024-wide passes halve the gate/copy op counts (amortizing per-op
        # overhead on DVE/ACT/GpSimd).  PSUM: 3 x [128,1024] cos (6 banks)
        # + 1 x [OUT,1024] outT (2 banks) = 8.
        cps_pool = ctx.enter_context(tc.tile_pool(name="cps", bufs=3, space="PSUM"))
        out_ps = ctx.enter_context(tc.tile_pool(name="outps", bufs=1, space="PSUM"))
        outT = out_ps.tile([OUT, 1024], FP32, tag="outT")

        LAG = 2
        for s in range(2):
            sl = slice(s * 1024, (s + 1) * 1024)
            o0 = s * 1024

            def emit_agg(q, gt):
                for j in range(2):
                    nc.tensor.matmul(
                        outT[:, j * 512 : (j + 1) * 512],
                        hrm[q][:],
                        gt[:, j * 512 : (j + 1) * 512],
                        start=(q == 0), stop=(q == NQ - 1),
                    )

            pend = []
            for q in range(NQ):
                cps = cps_pool.tile([128, 1024], FP32, tag="cps")
                for j in range(2):
                    nc.tensor.matmul(
                        cps[:, j * 512 : (j + 1) * 512],
                        hnT[:, q * 128 : (q + 1) * 128],
                        hnT[:, o0 + j * 512 : o0 + (j + 1) * 512],
                        start=True, stop=True,
                    )
                gt = gtp.tile([128, 1024], BF16, tag="gt")
                if q % 4 == 1:
                    csb = csp.tile([128, 1024], BF16, tag="csb")
                    nc.scalar.copy(csb[:], cps[:])
                    nc.gpsimd.tensor_mul(gt[:], csb[:], etbs[q][:, sl])
                elif q % 4 == 3:
                    csb = csp.tile([128, 1024], BF16, tag="csb")
                    nc.scalar.copy(csb[:], cps[:])
                    nc.vector.tensor_mul(gt[:], csb[:], etbs[q][:, sl])
                else:
                    nc.vector.tensor_mul(gt[:], cps[:], etbs[q][:, sl])
                pend.append((q, gt))
                if len(pend) > LAG:
                    emit_agg(*pend.pop(0))
            for item in pend:
                emit_agg(*item)

            nc.scalar.activation(out_sb[:, sl], outT[:], AF.Relu)
            nc.sync.dma_start(out[:, sl], out_sb[:, sl])

    nc.compile()
    return nc


_NC_CACHE = None


def _get_nc():
    global _NC_CACHE
    if _NC_CACHE is None:
        _NC_CACHE = build_nc()
    return _NC_CACHE


def make_in_maps(x, edge_weight, W, b):
    x = np.asarray(x, dtype=np.float32)
    edge_weight = np.asarray(edge_weight, dtype=np.float32)
    W = np.asarray(W, dtype=np.float32)
    b = np.asarray(b, dtype=np.float32)
    wt = np.ascontiguousarray(W.T)
    bias = np.ascontiguousarray(b.reshape(OUT, 1))
    in_maps = []
    for core in CORE_IDS:
        in_maps.append(
            {
                "et": np.ascontiguousarray(edge_weight[core].T).astype(
                    ml_dtypes.bfloat16
                ),
                "xt": np.ascontiguousarray(x[core].T),
                "wt": wt,
                "bias": bias,
            }
        )
    return in_maps


def kernel(x, edge_weight, W, b):
    nc = _get_nc()
    in_maps = make_in_maps(x, edge_weight, W, b)
    res = run_bass_kernel_spmd(nc, in_maps, core_ids=CORE_IDS)
    out = np.stack(
        [np.ascontiguousarray(res.results[i]["out"].T) for i in range(len(CORE_IDS))]
    )
    return out.astype(np.float32, copy=False)


# revision 27
# speedup vs baseline: 1.1358x; 1.1358x over previous
"""Trainium2 Bass kernel for AdaptConv-style GNN message passing.

Reference computation (per batch element b):
    h   = x @ W.T + b                       # [N, OUT]
    hn  = h / max(||h||_row, 1e-12)         # row-wise L2 normalize
    cos = hn @ hn.T                         # [N, N]
    out = relu((edge_weight * cos) @ h)     # [N, OUT]

Sharding: pure data-parallel over batch B=8 across the 8 NeuronCores
(no collectives).  Host-side layout preprocessing (part of the sharding
strategy): each core receives
    et   = edge_weight[b].T (bf16)  [N, N]  (so the gated matrix is produced
                                             directly in the [q, p] layout the
                                             aggregation matmul contracts over;
                                             bf16 halves the HBM stream and is
                                             well inside the accuracy budget)
    xt   = x[b].T               [IN, N]
    wt   = W.T                  [IN, OUT]
    bias = b.reshape(OUT, 1)
and returns outT = relu(out).T as [OUT, N]; the host transposes back.

On-chip dataflow per core (matmuls bf16, fp32 PSUM accumulation):
    hT[o, n]   = wt.T @ xt + bias          (TensorE + ScalarE bias)
    h_rm tiles = PE-transpose(hT)          (row-major h, agg stationary)
    norms      = Square+accum (ScalarE/VectorE) -> Sqrt/max/recip [128,16]
    hnT        = PE-transpose(h_rm * r)
    4 column passes x 16 q-bands (et bf16 fully SBUF-resident):
        cosT[q', p'] = hnT[:, q]^T @ hnT[:, sl]      (PE -> PSUM)
        gT           = et[q, sl] * cosT              (DVE / ACT+GpSimd / ACT+DVE)
        outT[:, sl] += h_rm[q]^T @ gT                (PE, PSUM accum, LAG behind)
    relu epilogue per pass (ScalarE) + DMA out.

PSUM: pass-local outT = 1 bank (2 alternating) + 6-deep cos pipeline = 8.
"""

import ml_dtypes
import numpy as np

import concourse.bass as bass
import concourse.mybir as mybir
import concourse.tile as tile
from concourse import bacc
from concourse.bass_utils import run_bass_kernel_spmd
from concourse.masks import make_identity

B, N, IN, OUT = 8, 2048, 128, 128
NQ = N // 128
NPC = N // 512
FP32 = mybir.dt.float32
BF16 = mybir.dt.bfloat16
AF = mybir.ActivationFunctionType
EPS = 1e-12

CORE_IDS = list(range(8))


def build_nc():
    """Build + compile the single-core Bass graph (same graph runs SPMD on 8 cores)."""
    from contextlib import ExitStack

    nc = bacc.Bacc("TRN2", target_bir_lowering=False, debug=False, num_devices=8)

    et = nc.dram_tensor("et", [N, N], BF16, kind="ExternalInput").ap()
    xt = nc.dram_tensor("xt", [IN, N], FP32, kind="ExternalInput").ap()
    wt = nc.dram_tensor("wt", [IN, OUT], FP32, kind="ExternalInput").ap()
    bias = nc.dram_tensor("bias", [OUT, 1], FP32, kind="ExternalInput").ap()
    out = nc.dram_tensor("out", [OUT, N], FP32, kind="ExternalOutput").ap()

    with tile.TileContext(nc) as tc, ExitStack() as ctx:
        singles = ctx.enter_context(tc.tile_pool(name="singles", bufs=1))
        # et stream pool FIRST so its SBUF range never aliases prologue
        # scratch (WAR deps would stall the stream behind the prologue).
        etp = ctx.enter_context(tc.tile_pool(name="etp", bufs=16))
        gtp = ctx.enter_context(tc.tile_pool(name="gtp", bufs=8))
        csp = ctx.enter_context(tc.tile_pool(name="csp", bufs=4))

        ident = singles.tile([128, 128], BF16, tag="ident")
        make_identity(nc, ident[:])

        hnT = singles.tile([128, N], BF16, tag="hnT")
        hrm = [
            singles.tile([128, OUT], BF16, tag=f"hrm{i}", name=f"hrm{i}")
            for i in range(NQ)
        ]
        out_sb = singles.tile([OUT, N], FP32, tag="out_sb")
        bias_sb = singles.tile([OUT, 1], FP32, tag="bias")
        s_acc = singles.tile([128, NQ], FP32, tag="s_acc")
        s_nrm = singles.tile([128, NQ], FP32, tag="s_nrm")
        s_max = singles.tile([128, NQ], FP32, tag="s_max")
        r_inv = singles.tile([128, NQ], FP32, tag="r_inv")

        # params first on the sync ring (xt heads the prologue critical
        # path), then the et stream queues right behind
        xt_f = singles.tile([IN, N], FP32, tag="xt_f")
        nc.sync.dma_start(xt_f[:], xt)
        wt_f = singles.tile([IN, OUT], FP32, tag="wt_f")
        nc.sync.dma_start(wt_f[:], wt)
        nc.sync.dma_start(bias_sb[:], bias)

        # et stream: all 16 bands prefetched into SBUF (bf16, 64KB/partition)
        etbs = []
        for q in range(NQ):
            etb = etp.tile([128, N], BF16, tag="etb", name=f"etb{q}")
            nc.sync.dma_start(etb[:], et[q * 128 : (q + 1) * 128, :])
            etbs.append(etb)

        # ---------------- prologue: h, norms, hn (scoped pools) ----------------
        with ExitStack() as pctx:
            pro = pctx.enter_context(tc.tile_pool(name="pro", bufs=2))
            ppsum = pctx.enter_context(tc.tile_pool(name="ppsum", bufs=2, space="PSUM"))

            # warm the ScalarE Sqrt activation table off the critical path
            dummy = pro.tile([1, 2], FP32, tag="dummy")
            nc.gpsimd.memset(dummy[:], 1.0)
            dummy2 = pro.tile([1, 2], FP32, tag="dummy2")
            nc.scalar.activation(dummy2[:], dummy[:], AF.Sqrt)

            wt_b = pro.tile([IN, OUT], BF16, tag="wt_b")
            nc.vector.tensor_copy(wt_b[:], wt_f[:])
            # cast + linear per 512-chunk so compute starts before the full
            # xt transfer lands
            xt_b = pro.tile([IN, N], BF16, tag="xt_b")
            hT = pro.tile([128, N], BF16, tag="hT")
            for c in range(N // 512):
                sl = slice(c * 512, (c + 1) * 512)
                nc.vector.tensor_copy(xt_b[:, sl], xt_f[:, sl])
                ps = ppsum.tile([OUT, 512], FP32, tag="hT_ps")
                nc.tensor.matmul(ps[:], wt_b[:], xt_b[:, sl], start=True, stop=True)
                # hT = psum + bias (per-partition bias along OUT)
                nc.scalar.activation(
                    hT[:, sl], ps[:], AF.Identity, bias=bias_sb[:], scale=1.0
                )

            # stage 1: transposes hT -> row-major h tiles; square+rowsum
            # (norm reductions split across ScalarE / VectorE)
            for i in range(NQ):
                tp = ppsum.tile([128, 128], BF16, tag="tp", bufs=4)
                nc.tensor.transpose(tp[:], hT[:, i * 128 : (i + 1) * 128], ident[:])
                nc.any.tensor_copy(hrm[i][:], tp[:])
                sq = pro.tile([128, OUT], BF16, tag="sq", bufs=4)
                if i % 2 == 0:
                    nc.scalar.activation(
                        sq[:], hrm[i][:], AF.Square, accum_out=s_acc[:, i : i + 1]
                    )
                else:
                    nc.vector.tensor_mul(sq[:], hrm[i][:], hrm[i][:])
                    nc.vector.tensor_reduce(
                        s_acc[:, i : i + 1], sq[:],
                        mybir.AxisListType.X, mybir.AluOpType.add,
                    )
            # stage 2: batched sqrt -> clamp -> reciprocal
            nc.scalar.activation(s_nrm[:], s_acc[:], AF.Sqrt)
            nc.vector.tensor_scalar_max(s_max[:], s_nrm[:], EPS)
            nc.vector.reciprocal(r_inv[:], s_max[:])

            # stage 3: hn tiles + transpose back into hnT
            for i in range(NQ):
                hn_i = pro.tile([128, OUT], BF16, tag="hn_i", bufs=4)
                nc.vector.tensor_scalar_mul(hn_i[:], hrm[i][:], r_inv[:, i : i + 1])
                tp2 = ppsum.tile([128, 128], BF16, tag="tp", bufs=4)
                nc.tensor.transpose(tp2[:], hn_i[:], ident[:])
                nc.any.tensor_copy(hnT[:, i * 128 : (i + 1) * 128], tp2[:])

        # ---------------- main loop: 4 column passes x 16 bands ----------------
        cps_pool = ctx.enter_context(tc.tile_pool(name="cps", bufs=6, space="PSUM"))
        out_ps = ctx.enter_context(tc.tile_pool(name="outps", bufs=1, space="PSUM"))
        outTs = [
            out_ps.tile([OUT, 512], FP32, tag=f"outT{i}", name=f"outT{i}")
            for i in range(2)
        ]

        LAG = 3
        for s in range(4):
            sl = slice(s * 512, (s + 1) * 512)
            ot = outTs[s % 2]

            def emit_agg(q, gt, ot=ot):
                nc.tensor.matmul(
                    ot[:], hrm[q][:], gt[:],
                    start=(q == 0), stop=(q == NQ - 1),
                )

            pend = []
            for q in range(NQ):
                cps = cps_pool.tile([128, 512], FP32, tag="cps")
                nc.tensor.matmul(
                    cps[:],
                    hnT[:, q * 128 : (q + 1) * 128],
                    hnT[:, sl],
                    start=True, stop=True,
                )
                gt = gtp.tile([128, 512], BF16, tag="gt")
                if q % 4 == 1:
                    csb = csp.tile([128, 512], BF16, tag="csb")
                    nc.scalar.copy(csb[:], cps[:])
                    nc.gpsimd.tensor_mul(gt[:], csb[:], etbs[q][:, sl])
                elif q % 4 == 3:
                    csb = csp.tile([128, 512], BF16, tag="csb")
                    nc.scalar.copy(csb[:], cps[:])
                    nc.vector.tensor_mul(gt[:], csb[:], etbs[q][:, sl])
                else:
                    nc.vector.tensor_mul(gt[:], cps[:], etbs[q][:, sl])
                pend.append((q, gt))
                if len(pend) > LAG:
                    emit_agg(*pend.pop(0))
            for item in pend:
                emit_agg(*item)

            nc.scalar.activation(out_sb[:, sl], ot[:], AF.Relu)
            nc.sync.dma_start(out[:, sl], out_sb[:, sl])

    nc.compile()
    return nc


_NC_CACHE = None


def _get_nc():
    global _NC_CACHE
    if _NC_CACHE is None:
        _NC_CACHE = build_nc()
    return _NC_CACHE


def make_in_maps(x, edge_weight, W, b):
    x = np.asarray(x, dtype=np.float32)
    edge_weight = np.asarray(edge_weight, dtype=np.float32)
    W = np.asarray(W, dtype=np.float32)
    b = np.asarray(b, dtype=np.float32)
    wt = np.ascontiguousarray(W.T)
    bias = np.ascontiguousarray(b.reshape(OUT, 1))
    in_maps = []
    for core in CORE_IDS:
        in_maps.append(
            {
                "et": np.ascontiguousarray(edge_weight[core].T).astype(
                    ml_dtypes.bfloat16
                ),
                "xt": np.ascontiguousarray(x[core].T),
                "wt": wt,
                "bias": bias,
            }
        )
    return in_maps


def kernel(x, edge_weight, W, b):
    nc = _get_nc()
    in_maps = make_in_maps(x, edge_weight, W, b)
    res = run_bass_kernel_spmd(nc, in_maps, core_ids=CORE_IDS)
    out = np.stack(
        [np.ascontiguousarray(res.results[i]["out"].T) for i in range(len(CORE_IDS))]
    )
    return out.astype(np.float32, copy=False)
